# Initial kernel scaffold
#
"""Trainium2 Bass kernel for nn_FAM_Deform: x1 + deform_conv(x1*x2), sharded
over 8 NeuronCores (batch x H-half).

Alpha-form decomposition (exact for |offsets| <= 1; validated vs reference):
  x   = x1 * x2   (zero-padded domain)
  off = conv3x3(x, W_off) + b_off -> dy_k, dx_k (9 taps)
  sampled_k = sum_alpha m^a_k (.) D^a(. + tap_k)      (9 alpha terms)
    alpha: 1 (D=x), py/ny/px/nx (first differences of x),
           py*px, py*nx, ny*px, ny*nx (second differences of x)
  out = x1 + b_dc + sum_k Wdc_k @ sampled_k

All adds are absorbed by PE PSUM accumulation: per 8-row block, one psum
chain per 432-col chunk runs 9 fused main+offset tap matmuls (combined
[64,82] weights) then 36 correction matmuls with [128,64] duplicated
weights against partition-paired masked maps.  DVE work per block is only
8 window difference-maps + 9 fused [128, 4*1296] mask multiplies; masks are
broadcast to 64 partitions by DMA (4 contiguous rows per tap half).

Rows are padded to width 162 (2 zero cols) so all shifts are flat offsets;
x window has a 3-row halo so difference maps are exact at shard edges.
"""

import numpy as np
import ml_dtypes

import concourse.bass as bass
import concourse.bacc as bacc
import concourse.tile as tile
from concourse import mybir
from concourse import bass_utils
from concourse.alu_op_type import AluOpType

F32 = mybir.dt.float32
BF16 = mybir.dt.bfloat16
AF = mybir.ActivationFunctionType

B, C, H, W = 4, 64, 160, 160
WP = W + 2              # padded row width
RIN = 86                # shard rows incl 3-row halo each side
ROUT = 80               # output rows per core
R = 8                   # output rows per block
NBLK = ROUT // R        # 10
N = R * WP              # 1296 flat elems per block
WIN = 12 * WP           # 1944: D-map window (rows -2..+9 rel to block)
CH = 432                # psum chunk (3 per block)
NCH = N // CH

TAPS = [(ky, kx) for ky in (-1, 0, 1) for kx in (-1, 0, 1)]


def _build_nc():
    nc = bacc.Bacc("TRN2", debug=False, num_devices=8)
    x1h = nc.dram_tensor("x1h", [C, RIN, W], F32, kind="ExternalInput")
    x2h = nc.dram_tensor("x2h", [C, RIN, W], F32, kind="ExternalInput")
    wcomb = nc.dram_tensor("wcomb", [C, 9 * 114], BF16, kind="ExternalInput")
    wdcdup = nc.dram_tensor("wdcdup", [128, 9 * 64], BF16, kind="ExternalInput")
    boff = nc.dram_tensor("boff", [18, 4], F32, kind="ExternalInput")
    bdc = nc.dram_tensor("bdc", [C, 1], F32, kind="ExternalInput")
    y = nc.dram_tensor("y", [C, ROUT, W], BF16, kind="ExternalOutput")

    with tile.TileContext(nc, num_cores=8) as tc:
        _kernel_body(nc, tc, x1h, x2h, wcomb, wdcdup, boff, bdc, y)
    nc.compile()
    return nc


def _flat(v, off, dims):
    """Manual AP into a tile view v at flat free offset off."""
    return bass.AP(v.tensor, v.offset + off, [list(v.ap[0])] + dims)


def _kernel_body(nc, tc, x1h, x2h, wcomb, wdcdup, boff, bdc, y):
    import contextlib
    ctx = contextlib.ExitStack()
    with ctx:
        const = ctx.enter_context(tc.tile_pool(name="const", bufs=1))
        xpool = ctx.enter_context(tc.tile_pool(name="xbuf", bufs=1))
        ldp = ctx.enter_context(tc.tile_pool(name="ld", bufs=2))
        dpool = ctx.enter_context(tc.tile_pool(name="dmaps", bufs=2))
        gpool = ctx.enter_context(tc.tile_pool(name="gmask", bufs=2))
        mpool = ctx.enter_context(tc.tile_pool(name="mtb", bufs=2, space="DRAM"))
        smp = ctx.enter_context(tc.tile_pool(name="smask", bufs=4))
        upool = ctx.enter_context(tc.tile_pool(name="ustage", bufs=4))
        outp = ctx.enter_context(tc.tile_pool(name="out", bufs=1))
        resp = ctx.enter_context(tc.tile_pool(name="resp", bufs=2))
        psp = ctx.enter_context(tc.tile_pool(name="ps", bufs=2, space="PSUM"))

        # ---- constants ----
        wcomb_t = const.tile([C, 9 * 114], BF16)
        nc.sync.dma_start(wcomb_t[:], wcomb[:])
        wdcdup_t = const.tile([128, 9 * 64], BF16)
        nc.sync.dma_start(wdcdup_t[:], wdcdup[:])
        boff_t = const.tile([18, 4], F32)
        nc.sync.dma_start(boff_t[:], boff[:])
        bdc_t = const.tile([C, 1], F32)
        nc.sync.dma_start(bdc_t[:], bdc[:])

        # ---- PE warm-up: dummy chain while x loads (ramps pstate) ----
        wps = psp.tile([64, 512], F32, tag="warm")
        for i in range(8):
            nc.tensor.matmul(wps[:, :], wcomb_t[:, 0:64], wcomb_t[:, 0:512],
                             start=(i == 0), stop=(i == 7))
        wscr = const.tile([1, 1], F32, tag="wscr")
        nc.vector.tensor_scalar_add(wscr[:], wps[0:1, 0:1], 0.0)

        # ---- x = x1 * x2 in padded-row bf16 layout ----
        xbuf = xpool.tile([C, RIN * WP], BF16)
        xv = xbuf[:]
        # zero only the 2 pad cols of each row
        nc.vector.memset(_flat(xv, W, [[WP, RIN], [1, 2]]), 0.0)
        rows_per_chunk = 8

        def xchunk(c):
            rc = c * rows_per_chunk
            if rc >= RIN:
                return
            nr = min(rows_per_chunk, RIN - rc)
            t1 = ldp.tile([C, nr * W], F32, tag="ld1", name=f"ld1_{c}")
            nc.scalar.dma_start(t1[:], x1h[:, rc:rc + nr, :])
            t2 = ldp.tile([C, nr * W], F32, tag="ld2", name=f"ld2_{c}")
            nc.scalar.dma_start(t2[:], x2h[:, rc:rc + nr, :])
            dst = _flat(xv, rc * WP, [[WP, nr], [1, W]])
            nc.vector.tensor_mul(dst, t1[:].rearrange("c (r w) -> c r w", w=W),
                                 t2[:].rearrange("c (r w) -> c r w", w=W))

        xchunk(0)
        xchunk(1)

        def dsubf(blk):
            base = (8 * blk + 1) * WP   # window row 0 = xbuf row 8b+1

            # ---- D maps: D4 [128, 4*WIN]; segs (Dy+;Dy-)(Dx+;Dx-)(D++;D+-)(D-+;D--)
            d4 = dpool.tile([128, 4 * WIN], BF16, tag="d4", name=f"d4_{blk}")
            d4t = d4[0:64, :]
            d4b = d4[64:128, :]

            def xw(off, n):
                return _flat(xv, base + off, [[1, n]])

            def dv(half, off, n):
                return _flat(half, off, [[1, n]])

            # first-order (rows 0..11 of the window)
            nc.vector.tensor_sub(dv(d4t, 0, WIN), xw(WP, WIN), xw(0, WIN))
            nc.gpsimd.tensor_sub(dv(d4b, 0, WIN), xw(-WP, WIN), xw(0, WIN))
            nc.vector.tensor_sub(dv(d4t, WIN, WIN), xw(1, WIN), xw(0, WIN))
            nc.gpsimd.tensor_sub(dv(d4b, WIN, WIN), xw(-1, WIN), xw(0, WIN))
            # zero Dx+ pad col 161 (only contaminated D entry)
            nc.vector.memset(_flat(d4t, WIN + W + 1, [[WP, 12], [1, 1]]), 0.0)
            # second-order from seg1 (valid rows 0..10 / 1..11; rows 1..10 used)
            n2 = 11 * WP
            nc.vector.tensor_sub(dv(d4t, 2 * WIN, n2),
                                 dv(d4t, WIN + WP, n2), dv(d4t, WIN, n2))
            nc.gpsimd.tensor_sub(dv(d4b, 2 * WIN, n2),
                                 dv(d4b, WIN + WP, n2), dv(d4b, WIN, n2))
            nc.vector.tensor_sub(dv(d4t, 3 * WIN + WP, n2),
                                 dv(d4t, WIN, n2), dv(d4t, WIN + WP, n2))
            nc.vector.tensor_sub(dv(d4b, 3 * WIN + WP, n2),
                                 dv(d4b, WIN, n2), dv(d4b, WIN + WP, n2))
            # rows never computed but grazed by shifted flat reads (pad cols)
            nc.vector.memset(_flat(d4[:], 2 * WIN + n2, [[1, WP]]), 0.0)
            nc.vector.memset(_flat(d4[:], 3 * WIN, [[1, WP]]), 0.0)
            return d4

        def prefix_front(blk):
            base = (8 * blk + 1) * WP

            # ---- fused main+off tap chains per chunk; off rows 64..82 ----
            pss = []
            for c0 in range(NCH):
                ps = psp.tile([128, CH], F32, tag=f"ps{c0}", name=f"ps{c0}_{blk}")
                for t, (ky, kx) in enumerate(TAPS):
                    rhs = _flat(xv, base + (2 + ky) * WP + kx + c0 * CH,
                                [[1, CH]])
                    nc.tensor.matmul(ps[0:114, :],
                                     wcomb_t[:, t * 114:(t + 1) * 114],
                                     rhs, start=(t == 0), stop=(t == 8))
                pss.append(ps)

            # ---- masks, relu straight from psum (psum rows 64:82 = [dy;dx],
            # rows 96:114 = [dx;dy]); all mask tiles live at partition base 0
            p1 = gpool.tile([18, N], BF16, tag="p1", name=f"p1_{blk}")  # py;px
            p2 = gpool.tile([18, N], BF16, tag="p2", name=f"p2_{blk}")  # px;py
            n1 = gpool.tile([18, N], BF16, tag="n1", name=f"n1_{blk}")  # ny;nx
            n2 = gpool.tile([18, N], BF16, tag="n2", name=f"n2_{blk}")  # nx;ny
            for c0 in range(NCH):
                cs = slice(c0 * CH, (c0 + 1) * CH)
                ps = pss[c0]
                nc.scalar.activation(p1[:, cs], ps[64:82, :], AF.Relu,
                                     bias=boff_t[:, 0:1])
                nc.scalar.activation(p2[:, cs], ps[96:114, :], AF.Relu,
                                     bias=boff_t[:, 1:2])
                nc.scalar.activation(n1[:, cs], ps[64:82, :], AF.Relu,
                                     scale=-1.0, bias=boff_t[:, 2:3])
                nc.scalar.activation(n2[:, cs], ps[96:114, :], AF.Relu,
                                     scale=-1.0, bias=boff_t[:, 3:4])
            # cross masks on Pool; same-base operands, row-paired products:
            # x1 = [pn; np], x2 = [pp; pp], x3 = [nn; nn]
            return pss, p1, p2, n1, n2

        def crossf(blk, fr):
            pss, p1, p2, n1, n2 = fr
            # cross masks on DVE right after the previous block's u-mul stream
            # xt: rows 0:18 = [pn;np], 32:41 = pp, 64:73 = nn
            xt = gpool.tile([128, N], BF16, tag="xt", name=f"xt_{blk}")
            nc.vector.tensor_mul(xt[0:18, :], p1[:], n2[:])
            nc.vector.tensor_mul(xt[32:41, :], p1[0:9, :], p2[0:9, :])
            nc.vector.tensor_mul(xt[64:73, :], n1[0:9, :], n2[0:9, :])
            return xt

        def permf(blk, fr, xt):
            pss, p1, p2, n1, n2 = fr
            # ---- permute to tap-major MT/MB [36, N] in DRAM: rows 4k+s
            # MT: (py, px, pp, np)   MB: (ny, nx, pn, nn)
            mt = mpool.tile([36, N], BF16, tag="mt", name=f"mt_{blk}")
            mb = mpool.tile([36, N], BF16, tag="mb", name=f"mb_{blk}")
            for m, eng, srcs in (
                    (mt, nc.scalar,
                     ((p1, 0), (p1, 9 * N), (xt, 32 * N), (xt, 9 * N))),
                    (mb, nc.scalar,
                     ((n1, 0), (n1, 9 * N), (xt, 0), (xt, 64 * N)))):
                mv = m[:]
                for s, (tile_, off_) in enumerate(srcs):
                    tv = tile_[:]
                    eng.dma_start(
                        bass.AP(mv.tensor, mv.offset + s * N,
                                [[4 * N, 9], [1, N]]),
                        bass.AP(tv.tensor, tv.offset + off_, [[N, 9], [1, N]]))
            # residual rows load early (no deps)
            resid = resp.tile([C, R * W], F32, tag="resid", name=f"res_{blk}")
            nc.scalar.dma_start(resid[:], x1h[:, 8 * blk + 3:8 * blk + 3 + R, :])
            return mt, mb, resid

        def suffix_u(blk, st):
            d4, pss, mt, mb, resid = st
            # ---- per tap: broadcast masks, fused mask-mul, 4 correction mms
            for t, (ky, kx) in enumerate(TAPS):
                sm = smp.tile([128, 4 * N], BF16, tag="sm", name=f"sm{t}_{blk}")
                for half, m in ((0, mt), (1, mb)):
                    mv = m[:]
                    src = bass.AP(mv.tensor, mv.offset + 4 * t * N,
                                  [[1, 1], [0, 64], [1, 4 * N]])
                    nc.sync.dma_start(sm[64 * half:64 * (half + 1), :], src)
                u = upool.tile([128, 4 * N], BF16, tag="u", name=f"u{t}_{blk}")
                nc.vector.tensor_mul(
                    u[:].rearrange("p (s n) -> p s n", n=N),
                    sm[:].rearrange("p (s n) -> p s n", n=N),
                    _flat(d4[:], (2 + ky) * WP + kx, [[WIN, 4], [1, N]]))
                for seg in range(4):
                    for c0 in range(NCH):
                        nc.tensor.matmul(
                            pss[c0][0:64, :],
                            wdcdup_t[:, t * 64:(t + 1) * 64],
                            u[:, seg * N + c0 * CH: seg * N + c0 * CH + CH],
                            start=False, stop=(t == 8 and seg == 3),
                            skip_group_check=True)

        def suffix_out(blk, st):
            d4, pss, mt, mb, resid = st
            # ---- out = ps + b_dc (DVE, frees psum) + x1 (Pool), store
            out_sb = outp.tile([C, N], BF16, tag="osb", name=f"osb_{blk}")
            for c0 in range(NCH):
                nc.vector.tensor_scalar_add(out_sb[:, c0 * CH:(c0 + 1) * CH],
                                            pss[c0][0:64, :], bdc_t[:])
            ost = outp.tile([C, R * W], BF16, tag="ost", name=f"ost_{blk}")
            nc.gpsimd.tensor_add(
                ost[:].rearrange("c (r w) -> c r w", w=W),
                _flat(out_sb[:], 0, [[WP, R], [1, W]]),
                resid[:].rearrange("c (r w) -> c r w", w=W))
            nc.gpsimd.dma_start(y[:, 8 * blk:8 * blk + R, :], ost[:])

        fr = prefix_front(0)
        xt0 = crossf(0, fr)
        mt0, mb0, res0 = permf(0, fr, xt0)
        st = (dsubf(0), fr[0], mt0, mb0, res0)
        for blk in range(1, NBLK):
            xchunk(blk + 1)
            fr = prefix_front(blk)
            suffix_u(blk - 1, st)
            xt = crossf(blk, fr)
            suffix_out(blk - 1, st)
            mt, mb, resid = permf(blk, fr, xt)
            st = (dsubf(blk), fr[0], mt, mb, resid)
        suffix_u(NBLK - 1, st)
        suffix_out(NBLK - 1, st)


_NC_CACHE = None


def _get_nc():
    global _NC_CACHE
    if _NC_CACHE is None:
        _NC_CACHE = _build_nc()
    return _NC_CACHE


def kernel(x1, x2, W_off, b_off, W_dc, b_dc):
    x1 = np.asarray(x1, np.float32)
    x2 = np.asarray(x2, np.float32)
    W_off = np.asarray(W_off, np.float32)
    b_off = np.asarray(b_off, np.float32)
    W_dc = np.asarray(W_dc, np.float32)
    b_dc = np.asarray(b_dc, np.float32)

    # combined per-tap weights [C, 9*114]:
    # [wdct_k | dy;dx offsets | zeros | dx;dy offsets]
    wcomb = np.zeros((C, 9 * 114), np.float32)
    wdcdup = np.empty((128, 9 * 64), np.float32)
    wdc9 = W_dc.reshape(C, C, 9)  # [o, c, k] with k = 3*(ky+1)+(kx+1)
    for t, (ky, kx) in enumerate(TAPS):
        wdct_k = wdc9[:, :, t].T  # [c, o]
        b0 = t * 114
        wcomb[:, b0:b0 + 64] = wdct_k
        for j in range(9):
            wdy = W_off[2 * j, :, ky + 1, kx + 1]
            wdx = W_off[2 * j + 1, :, ky + 1, kx + 1]
            wcomb[:, b0 + 64 + j] = wdy
            wcomb[:, b0 + 73 + j] = wdx
            wcomb[:, b0 + 96 + j] = wdx
            wcomb[:, b0 + 105 + j] = wdy
        wdcdup[0:64, t * 64:(t + 1) * 64] = wdct_k
        wdcdup[64:128, t * 64:(t + 1) * 64] = wdct_k
    wcomb = wcomb.astype(ml_dtypes.bfloat16)
    wdcdup = wdcdup.astype(ml_dtypes.bfloat16)
    bdy, bdx = b_off[0::2], b_off[1::2]
    boffr = np.zeros((18, 4), np.float32)
    boffr[0:9, 0], boffr[9:18, 0] = bdy, bdx
    boffr[0:9, 1], boffr[9:18, 1] = bdx, bdy
    boffr[0:9, 2], boffr[9:18, 2] = -bdy, -bdx
    boffr[0:9, 3], boffr[9:18, 3] = -bdx, -bdy
    bdct = b_dc.reshape(C, 1).astype(np.float32)

    in_maps = []
    for i in range(8):
        b, half = i // 2, i % 2
        lo = half * 80 - 3
        x1p = np.zeros((C, RIN, W), np.float32)
        x2p = np.zeros((C, RIN, W), np.float32)
        g0, g1 = max(0, lo), min(H, lo + RIN)
        x1p[:, g0 - lo:g1 - lo] = x1[b][:, g0:g1]
        x2p[:, g0 - lo:g1 - lo] = x2[b][:, g0:g1]
        in_maps.append({
            "x1h": np.ascontiguousarray(x1p),
            "x2h": np.ascontiguousarray(x2p),
            "wcomb": wcomb, "wdcdup": wdcdup, "boff": boffr, "bdc": bdct,
        })

    nc = _get_nc()
    res = bass_utils.run_bass_kernel_spmd(nc, in_maps, core_ids=list(range(8)))
    out = np.empty((B, C, H, W), np.float32)
    for i in range(8):
        b, half = i // 2, i % 2
        out[b, :, half * 80:(half + 1) * 80, :] = \
            res.results[i]["y"].astype(np.float32)
    return out


if __name__ == "__main__":
    rng = np.random.RandomState(0)
    inputs = {
        "x1": rng.randn(B, C, H, W).astype(np.float32),
        "x2": rng.randn(B, C, H, W).astype(np.float32),
        "W_off": (rng.randn(18, C, 3, 3) * 0.004).astype(np.float32),
        "b_off": np.zeros(18, np.float32),
        "W_dc": (rng.randn(C, C, 3, 3) / 24).astype(np.float32),
        "b_dc": np.zeros(C, np.float32),
    }
    out = kernel(**inputs)
    print("kernel ran, out shape", out.shape)



# revision 1
# speedup vs baseline: 1.0286x; 1.0286x over previous
"""Trainium2 Bass kernel for nn_FAM_Deform: x1 + deform_conv(x1*x2), sharded
over 8 NeuronCores (batch x H-half).

Alpha-form decomposition (exact for |offsets| <= 1; validated vs reference):
  x   = x1 * x2   (zero-padded domain)
  off = conv3x3(x, W_off) + b_off -> dy_k, dx_k (9 taps)
  sampled_k = sum_alpha m^a_k (.) D^a(. + tap_k)      (9 alpha terms)
    alpha: 1 (D=x), py/ny/px/nx (first differences of x),
           py*px, py*nx, ny*px, ny*nx (second differences of x)
  out = x1 + b_dc + sum_k Wdc_k @ sampled_k

All adds are absorbed by PE PSUM accumulation: per 8-row block, one psum
chain per 432-col chunk runs 9 fused main+offset tap matmuls (combined
[64,82] weights) then 36 correction matmuls with [128,64] duplicated
weights against partition-paired masked maps.  DVE work per block is only
8 window difference-maps + 9 fused [128, 4*1296] mask multiplies; masks are
broadcast to 64 partitions by DMA (4 contiguous rows per tap half).

Rows are padded to width 162 (2 zero cols) so all shifts are flat offsets;
x window has a 3-row halo so difference maps are exact at shard edges.
"""

import numpy as np
import ml_dtypes

import concourse.bass as bass
import concourse.bacc as bacc
import concourse.tile as tile
from concourse import mybir
from concourse import bass_utils
from concourse.alu_op_type import AluOpType

F32 = mybir.dt.float32
BF16 = mybir.dt.bfloat16
AF = mybir.ActivationFunctionType

B, C, H, W = 4, 64, 160, 160
WP = W + 2              # padded row width
RIN = 86                # shard rows incl 3-row halo each side
ROUT = 80               # output rows per core
R = 8                   # output rows per block
NBLK = ROUT // R        # 10
N = R * WP              # 1296 flat elems per block
WIN = 12 * WP           # 1944: D-map window (rows -2..+9 rel to block)
CH = 432                # psum chunk (3 per block)
NCH = N // CH

TAPS = [(ky, kx) for ky in (-1, 0, 1) for kx in (-1, 0, 1)]


def _build_nc():
    nc = bacc.Bacc("TRN2", debug=False, num_devices=8)
    x1h = nc.dram_tensor("x1h", [C, RIN, W], F32, kind="ExternalInput")
    x2h = nc.dram_tensor("x2h", [C, RIN, W], F32, kind="ExternalInput")
    wcomb = nc.dram_tensor("wcomb", [C, 9 * 114], BF16, kind="ExternalInput")
    wdcdup = nc.dram_tensor("wdcdup", [128, 9 * 64], BF16, kind="ExternalInput")
    boff = nc.dram_tensor("boff", [18, 4], F32, kind="ExternalInput")
    bdc = nc.dram_tensor("bdc", [C, 1], F32, kind="ExternalInput")
    y = nc.dram_tensor("y", [C, ROUT, W], BF16, kind="ExternalOutput")

    with tile.TileContext(nc, num_cores=8) as tc:
        _kernel_body(nc, tc, x1h, x2h, wcomb, wdcdup, boff, bdc, y)
    nc.compile()
    return nc


def _flat(v, off, dims):
    """Manual AP into a tile view v at flat free offset off."""
    return bass.AP(v.tensor, v.offset + off, [list(v.ap[0])] + dims)


def _kernel_body(nc, tc, x1h, x2h, wcomb, wdcdup, boff, bdc, y):
    import contextlib
    ctx = contextlib.ExitStack()
    with ctx:
        const = ctx.enter_context(tc.tile_pool(name="const", bufs=1))
        xpool = ctx.enter_context(tc.tile_pool(name="xbuf", bufs=1))
        ldp = ctx.enter_context(tc.tile_pool(name="ld", bufs=2))
        dpool = ctx.enter_context(tc.tile_pool(name="dmaps", bufs=2))
        gpool = ctx.enter_context(tc.tile_pool(name="gmask", bufs=2))
        mpool = ctx.enter_context(tc.tile_pool(name="mtb", bufs=2, space="DRAM"))
        smp = ctx.enter_context(tc.tile_pool(name="smask", bufs=4))
        upool = ctx.enter_context(tc.tile_pool(name="ustage", bufs=4))
        outp = ctx.enter_context(tc.tile_pool(name="out", bufs=1))
        resp = ctx.enter_context(tc.tile_pool(name="resp", bufs=2))
        psp = ctx.enter_context(tc.tile_pool(name="ps", bufs=2, space="PSUM"))

        # ---- constants ----
        wcomb_t = const.tile([C, 9 * 114], BF16)
        nc.sync.dma_start(wcomb_t[:], wcomb[:])
        wdcdup_t = const.tile([128, 9 * 64], BF16)
        nc.sync.dma_start(wdcdup_t[:], wdcdup[:])
        boff_t = const.tile([18, 4], F32)
        nc.sync.dma_start(boff_t[:], boff[:])
        bdc_t = const.tile([C, 1], F32)
        nc.sync.dma_start(bdc_t[:], bdc[:])

        # ---- PE warm-up: dummy chain while x loads (ramps pstate) ----
        wps = psp.tile([64, 512], F32, tag="warm")
        for i in range(8):
            nc.tensor.matmul(wps[:, :], wcomb_t[:, 0:64], wcomb_t[:, 0:512],
                             start=(i == 0), stop=(i == 7))
        wscr = const.tile([1, 1], F32, tag="wscr")
        nc.vector.tensor_scalar_add(wscr[:], wps[0:1, 0:1], 0.0)

        # ---- x = x1 * x2 in padded-row bf16 layout ----
        xbuf = xpool.tile([C, RIN * WP], BF16)
        xv = xbuf[:]
        # zero only the 2 pad cols of each row
        nc.vector.memset(_flat(xv, W, [[WP, RIN], [1, 2]]), 0.0)
        rows_per_chunk = 8

        def xchunk(c):
            rc = c * rows_per_chunk
            if rc >= RIN:
                return
            nr = min(rows_per_chunk, RIN - rc)
            t1 = ldp.tile([C, nr * W], F32, tag="ld1", name=f"ld1_{c}")
            nc.scalar.dma_start(t1[:], x1h[:, rc:rc + nr, :])
            t2 = ldp.tile([C, nr * W], F32, tag="ld2", name=f"ld2_{c}")
            nc.scalar.dma_start(t2[:], x2h[:, rc:rc + nr, :])
            dst = _flat(xv, rc * WP, [[WP, nr], [1, W]])
            nc.vector.tensor_mul(dst, t1[:].rearrange("c (r w) -> c r w", w=W),
                                 t2[:].rearrange("c (r w) -> c r w", w=W))

        xchunk(0)
        xchunk(1)

        def dsubf(blk):
            base = (8 * blk + 1) * WP   # window row 0 = xbuf row 8b+1

            # ---- D maps: D4 [128, 4*WIN]; segs (Dy+;Dy-)(Dx+;Dx-)(D++;D+-)(D-+;D--)
            d4 = dpool.tile([128, 4 * WIN], BF16, tag="d4", name=f"d4_{blk}")
            d4t = d4[0:64, :]
            d4b = d4[64:128, :]

            def xw(off, n):
                return _flat(xv, base + off, [[1, n]])

            def dv(half, off, n):
                return _flat(half, off, [[1, n]])

            # first-order (rows 0..11 of the window)
            nc.vector.tensor_sub(dv(d4t, 0, WIN), xw(WP, WIN), xw(0, WIN))
            nc.gpsimd.tensor_sub(dv(d4b, 0, WIN), xw(-WP, WIN), xw(0, WIN))
            nc.vector.tensor_sub(dv(d4t, WIN, WIN), xw(1, WIN), xw(0, WIN))
            nc.gpsimd.tensor_sub(dv(d4b, WIN, WIN), xw(-1, WIN), xw(0, WIN))
            # zero Dx+ pad col 161 (only contaminated D entry)
            nc.vector.memset(_flat(d4t, WIN + W + 1, [[WP, 12], [1, 1]]), 0.0)
            # second-order from seg1 (valid rows 0..10 / 1..11; rows 1..10 used)
            n2 = 11 * WP
            nc.vector.tensor_sub(dv(d4t, 2 * WIN, n2),
                                 dv(d4t, WIN + WP, n2), dv(d4t, WIN, n2))
            nc.gpsimd.tensor_sub(dv(d4b, 2 * WIN, n2),
                                 dv(d4b, WIN + WP, n2), dv(d4b, WIN, n2))
            nc.vector.tensor_sub(dv(d4t, 3 * WIN + WP, n2),
                                 dv(d4t, WIN, n2), dv(d4t, WIN + WP, n2))
            nc.vector.tensor_sub(dv(d4b, 3 * WIN + WP, n2),
                                 dv(d4b, WIN, n2), dv(d4b, WIN + WP, n2))
            # rows never computed but grazed by shifted flat reads (pad cols)
            nc.vector.memset(_flat(d4[:], 2 * WIN + n2, [[1, WP]]), 0.0)
            nc.vector.memset(_flat(d4[:], 3 * WIN, [[1, WP]]), 0.0)
            return d4

        def prefix_front(blk):
            base = (8 * blk + 1) * WP

            # ---- fused main+off tap chains per chunk; off rows 64..82 ----
            pss = []
            for c0 in range(NCH):
                ps = psp.tile([128, CH], F32, tag=f"ps{c0}", name=f"ps{c0}_{blk}")
                for t, (ky, kx) in enumerate(TAPS):
                    rhs = _flat(xv, base + (2 + ky) * WP + kx + c0 * CH,
                                [[1, CH]])
                    nc.tensor.matmul(ps[0:114, :],
                                     wcomb_t[:, t * 114:(t + 1) * 114],
                                     rhs, start=(t == 0), stop=(t == 8))
                pss.append(ps)

            # ---- masks, relu straight from psum (psum rows 64:82 = [dy;dx],
            # rows 96:114 = [dx;dy]); all mask tiles live at partition base 0
            p1 = gpool.tile([18, N], BF16, tag="p1", name=f"p1_{blk}")  # py;px
            p2 = gpool.tile([18, N], BF16, tag="p2", name=f"p2_{blk}")  # px;py
            n1 = gpool.tile([18, N], BF16, tag="n1", name=f"n1_{blk}")  # ny;nx
            n2 = gpool.tile([18, N], BF16, tag="n2", name=f"n2_{blk}")  # nx;ny
            for c0 in range(NCH):
                cs = slice(c0 * CH, (c0 + 1) * CH)
                ps = pss[c0]
                nc.scalar.activation(p1[:, cs], ps[64:82, :], AF.Relu,
                                     bias=boff_t[:, 0:1])
                nc.scalar.activation(p2[:, cs], ps[96:114, :], AF.Relu,
                                     bias=boff_t[:, 1:2])
                nc.scalar.activation(n1[:, cs], ps[64:82, :], AF.Relu,
                                     scale=-1.0, bias=boff_t[:, 2:3])
                nc.scalar.activation(n2[:, cs], ps[96:114, :], AF.Relu,
                                     scale=-1.0, bias=boff_t[:, 3:4])
            # cross masks on Pool; same-base operands, row-paired products:
            # x1 = [pn; np], x2 = [pp; pp], x3 = [nn; nn]
            return pss, p1, p2, n1, n2

        def crossf(blk, fr):
            pss, p1, p2, n1, n2 = fr
            # cross masks on DVE right after the previous block's u-mul stream
            # xt: rows 0:18 = [pn;np], 32:41 = pp, 64:73 = nn
            xt = gpool.tile([128, N], BF16, tag="xt", name=f"xt_{blk}")
            nc.vector.tensor_mul(xt[0:18, :], p1[:], n2[:])
            nc.vector.tensor_mul(xt[32:41, :], p1[0:9, :], p2[0:9, :])
            nc.vector.tensor_mul(xt[64:73, :], n1[0:9, :], n2[0:9, :])
            return xt

        def permf(blk, fr, xt):
            pss, p1, p2, n1, n2 = fr
            # ---- permute to tap-major MT/MB [36, N] in DRAM: rows 4k+s
            # MT: (py, px, pp, np)   MB: (ny, nx, pn, nn)
            mt = mpool.tile([36, N], BF16, tag="mt", name=f"mt_{blk}")
            mb = mpool.tile([36, N], BF16, tag="mb", name=f"mb_{blk}")
            for m, eng, srcs in (
                    (mt, nc.scalar,
                     ((p1, 0), (p1, 9 * N), (xt, 32 * N), (xt, 9 * N))),
                    (mb, nc.scalar,
                     ((n1, 0), (n1, 9 * N), (xt, 0), (xt, 64 * N)))):
                mv = m[:]
                for s, (tile_, off_) in enumerate(srcs):
                    tv = tile_[:]
                    eng.dma_start(
                        bass.AP(mv.tensor, mv.offset + s * N,
                                [[4 * N, 9], [1, N]]),
                        bass.AP(tv.tensor, tv.offset + off_, [[N, 9], [1, N]]))
            # residual rows load early (no deps)
            resid = resp.tile([C, R * W], F32, tag="resid", name=f"res_{blk}")
            nc.scalar.dma_start(resid[:], x1h[:, 8 * blk + 3:8 * blk + 3 + R, :])
            return mt, mb, resid

        def suffix_u(blk, st):
            d4, pss, mt, mb, resid = st
            # ---- per tap: broadcast masks, fused mask-mul, 4 correction mms
            for t, (ky, kx) in enumerate(TAPS):
                sm = smp.tile([128, 4 * N], BF16, tag="sm", name=f"sm{t}_{blk}")
                for half, m in ((0, mt), (1, mb)):
                    mv = m[:]
                    src = bass.AP(mv.tensor, mv.offset + 4 * t * N,
                                  [[1, 1], [0, 64], [1, 4 * N]])
                    nc.sync.dma_start(sm[64 * half:64 * (half + 1), :], src)
                u = upool.tile([128, 4 * N], BF16, tag="u", name=f"u{t}_{blk}")
                nc.vector.tensor_mul(
                    u[:].rearrange("p (s n) -> p s n", n=N),
                    sm[:].rearrange("p (s n) -> p s n", n=N),
                    _flat(d4[:], (2 + ky) * WP + kx, [[WIN, 4], [1, N]]))
                for seg in range(4):
                    for c0 in range(NCH):
                        nc.tensor.matmul(
                            pss[c0][0:64, :],
                            wdcdup_t[:, t * 64:(t + 1) * 64],
                            u[:, seg * N + c0 * CH: seg * N + c0 * CH + CH],
                            start=False, stop=(t == 8 and seg == 3),
                            skip_group_check=True)

        def suffix_out(blk, st):
            d4, pss, mt, mb, resid = st
            # ---- out = ps + b_dc (DVE, frees psum) + x1 (Pool), store
            out_sb = outp.tile([C, N], BF16, tag="osb", name=f"osb_{blk}")
            for c0 in range(NCH):
                nc.vector.tensor_scalar_add(out_sb[:, c0 * CH:(c0 + 1) * CH],
                                            pss[c0][0:64, :], bdc_t[:])
            ost = outp.tile([C, R * W], BF16, tag="ost", name=f"ost_{blk}")
            nc.gpsimd.tensor_add(
                ost[:].rearrange("c (r w) -> c r w", w=W),
                _flat(out_sb[:], 0, [[WP, R], [1, W]]),
                resid[:].rearrange("c (r w) -> c r w", w=W))
            nc.gpsimd.dma_start(y[:, 8 * blk:8 * blk + R, :], ost[:])

        fr = prefix_front(0)
        xt0 = crossf(0, fr)
        mt0, mb0, res0 = permf(0, fr, xt0)
        st = (dsubf(0), fr[0], mt0, mb0, res0)
        for blk in range(1, NBLK):
            xchunk(blk + 1)
            fr = prefix_front(blk)
            suffix_u(blk - 1, st)
            xt = crossf(blk, fr)
            suffix_out(blk - 1, st)
            mt, mb, resid = permf(blk, fr, xt)
            st = (dsubf(blk), fr[0], mt, mb, resid)
        suffix_u(NBLK - 1, st)
        suffix_out(NBLK - 1, st)


_NC_CACHE = None


def _get_nc():
    global _NC_CACHE
    if _NC_CACHE is None:
        _NC_CACHE = _build_nc()
    return _NC_CACHE


def kernel(x1, x2, W_off, b_off, W_dc, b_dc):
    x1 = np.asarray(x1, np.float32)
    x2 = np.asarray(x2, np.float32)
    W_off = np.asarray(W_off, np.float32)
    b_off = np.asarray(b_off, np.float32)
    W_dc = np.asarray(W_dc, np.float32)
    b_dc = np.asarray(b_dc, np.float32)

    # combined per-tap weights [C, 9*114]:
    # [wdct_k | dy;dx offsets | zeros | dx;dy offsets]
    wcomb = np.zeros((C, 9 * 114), np.float32)
    wdcdup = np.empty((128, 9 * 64), np.float32)
    wdc9 = W_dc.reshape(C, C, 9)  # [o, c, k] with k = 3*(ky+1)+(kx+1)
    for t, (ky, kx) in enumerate(TAPS):
        wdct_k = wdc9[:, :, t].T  # [c, o]
        b0 = t * 114
        wcomb[:, b0:b0 + 64] = wdct_k
        for j in range(9):
            wdy = W_off[2 * j, :, ky + 1, kx + 1]
            wdx = W_off[2 * j + 1, :, ky + 1, kx + 1]
            wcomb[:, b0 + 64 + j] = wdy
            wcomb[:, b0 + 73 + j] = wdx
            wcomb[:, b0 + 96 + j] = wdx
            wcomb[:, b0 + 105 + j] = wdy
        wdcdup[0:64, t * 64:(t + 1) * 64] = wdct_k
        wdcdup[64:128, t * 64:(t + 1) * 64] = wdct_k
    wcomb = wcomb.astype(ml_dtypes.bfloat16)
    wdcdup = wdcdup.astype(ml_dtypes.bfloat16)
    bdy, bdx = b_off[0::2], b_off[1::2]
    boffr = np.zeros((18, 4), np.float32)
    boffr[0:9, 0], boffr[9:18, 0] = bdy, bdx
    boffr[0:9, 1], boffr[9:18, 1] = bdx, bdy
    boffr[0:9, 2], boffr[9:18, 2] = -bdy, -bdx
    boffr[0:9, 3], boffr[9:18, 3] = -bdx, -bdy
    bdct = b_dc.reshape(C, 1).astype(np.float32)

    in_maps = []
    for i in range(8):
        b, half = i // 2, i % 2
        lo = half * 80 - 3
        x1p = np.zeros((C, RIN, W), np.float32)
        x2p = np.zeros((C, RIN, W), np.float32)
        g0, g1 = max(0, lo), min(H, lo + RIN)
        x1p[:, g0 - lo:g1 - lo] = x1[b][:, g0:g1]
        x2p[:, g0 - lo:g1 - lo] = x2[b][:, g0:g1]
        in_maps.append({
            "x1h": np.ascontiguousarray(x1p),
            "x2h": np.ascontiguousarray(x2p),
            "wcomb": wcomb, "wdcdup": wdcdup, "boff": boffr, "bdc": bdct,
        })

    nc = _get_nc()
    res = bass_utils.run_bass_kernel_spmd(nc, in_maps, core_ids=list(range(8)))
    out = np.empty((B, C, H, W), np.float32)
    for i in range(8):
        b, half = i // 2, i % 2
        out[b, :, half * 80:(half + 1) * 80, :] = \
            res.results[i]["y"].astype(np.float32)
    return out


if __name__ == "__main__":
    rng = np.random.RandomState(0)
    inputs = {
        "x1": rng.randn(B, C, H, W).astype(np.float32),
        "x2": rng.randn(B, C, H, W).astype(np.float32),
        "W_off": (rng.randn(18, C, 3, 3) * 0.004).astype(np.float32),
        "b_off": np.zeros(18, np.float32),
        "W_dc": (rng.randn(C, C, 3, 3) / 24).astype(np.float32),
        "b_dc": np.zeros(C, np.float32),
    }
    out = kernel(**inputs)
    print("kernel ran, out shape", out.shape)

